# revision 40
# baseline (speedup 1.0000x reference)
"""Trainium2 Bass kernel for MultiHeadPosAttn (attention + BN + FFN + BN).

Sharding: data-parallel over batch across 8 NeuronCores (2 images/core).
BatchNorm batch statistics are combined with tiny (1KB) AllReduces.

Math notes (verified exactly equivalent to the reference):
  - bk cancels in softmax (adds a per-query constant to every logit row).
  - bv cancels in BN1 (per-channel constant shift; softmax rows sum to 1).
  - b2 cancels in BN2 (per-channel constant shift).
  - PReLU(y) = Lrelu(y) with alpha = a (ACT supports a slope parameter).
  - softmax needs no max-subtraction: |logits| <= ~66 so exp() stays in
    fp32 range (max ~3e28 << 3.4e38).
Softmax denominator comes from an extra all-ones column in each head's
V^T block, so the attention matmul also produces sum_k(P) per query.

Collective scheduling (the previous version lost ~86us to two exposed
AllReduce waits): each BN's stats are now reduced with TWO collectives,
one per 128-channel chunk.  The first fires at the halfway point of the
producing phase (mid-attention for BN1, after W2 chunk 0 for BN2) so its
latency hides entirely under compute; the second fires at the end and is
covered by (a) the other chunk's scale/shift finish + apply, (b) partial
W1 accumulation over the already-available channel chunk (BN1), or the
other chunk's apply + output DMA (BN2).  Output is written f16 and
upcast host-side (rel tolerance is 2e-2; f16 rounds at 5e-4).
"""

import numpy as np

import concourse.bass as bass
import concourse.bacc as bacc
import concourse.tile as tile
from concourse import mybir
from concourse import bass_utils

F32 = mybir.dt.float32
BF16 = mybir.dt.bfloat16
F16 = mybir.dt.float16

B, C, HH, WW = 16, 256, 32, 32
N = HH * WW              # 1024 spatial positions
NH, DH = 4, 64           # heads, head dim
DFF = 4 * C              # 1024
EPS = 1e-5
NCORES = 8
BL = B // NCORES         # 2 images per core
NCH = C // 128           # 2 channel chunks of 128
NFC = DFF // 128         # 8 ffn chunks
NNC = N // 128           # 8 position chunks


def _build(a_slope: float):
    nc = bacc.Bacc("TRN2", target_bir_lowering=False, debug=False,
                   num_devices=NCORES)

    # all big inputs are pre-packed host-side into [128, free] partition-major
    # layouts so every load DMA is 128 descriptors of >=512B contiguous rows
    x_d = nc.dram_tensor("x", [BL, 128, NCH * N], F16, kind="ExternalInput")
    wq_d = nc.dram_tensor("wqT", [128, NCH * C], F16, kind="ExternalInput")
    wk_d = nc.dram_tensor("wkT", [128, NCH * C], F16, kind="ExternalInput")
    wv_d = nc.dram_tensor("wvT", [128, NCH * NH * DH], F16, kind="ExternalInput")
    w1_d = nc.dram_tensor("w1T", [128, NCH * DFF], F16, kind="ExternalInput")
    w2_d = nc.dram_tensor("w2T", [128, NFC * C], F16, kind="ExternalInput")
    # bq(2) | b1(8) | gamma(2) | beta(2) packed per partition
    sm_d = nc.dram_tensor("sm", [128, 14], F32, kind="ExternalInput")
    out_d = nc.dram_tensor("out", [BL, C, N], F16, kind="ExternalOutput")
    snk_d = nc.dram_tensor("snk", [4], F32, kind="ExternalOutput")

    with tile.TileContext(nc) as tc:
        _emit(tc, a_slope,
              x_d=x_d, wq_d=wq_d, wk_d=wk_d, wv_d=wv_d,
              w1_d=w1_d, w2_d=w2_d, sm_d=sm_d, out_d=out_d, snk_d=snk_d)
    nc.compile()
    return nc


def _emit(tc, a_slope, *, x_d, wq_d, wk_d, wv_d, w1_d, w2_d, sm_d, out_d,
          snk_d):
    nc = tc.nc
    from contextlib import ExitStack

    ctx = ExitStack()
    with ctx:
        const = ctx.enter_context(tc.tile_pool(name="const", bufs=1))
        data = ctx.enter_context(tc.tile_pool(name="data", bufs=1))
        work = ctx.enter_context(tc.tile_pool(name="work", bufs=1))
        dram = ctx.enter_context(tc.tile_pool(name="dram", bufs=1, space="DRAM"))

        # ---- warm-up collectives fire FIRST so the one-time CC channel
        # setup (and any cross-core launch skew) drains before the real
        # BN collectives need the stream.  Both the AllReduce AND the
        # AllGather paths are warmed (the runtime sets their channels up
        # separately; bn1_1/bn2 use AllGather) ----
        warm_sb = const.tile([1, 64], F32, name="warm_sb")
        nc.vector.memset(warm_sb, 0.0)
        w_in = dram.tile([64], F32, name="warm_in", tag="warm_in")
        w_out = dram.tile([64], F32, name="warm_out", tag="warm_out",
                          addr_space="Shared")
        wg_in = dram.tile([64], F32, name="warmg_in", tag="warmg_in")
        wg_out = dram.tile([64 * NCORES], F32, name="warmg_out",
                           tag="warmg_out", addr_space="Shared")
        nc.sync.dma_start(out=w_in.unsqueeze(0), in_=warm_sb)
        nc.sync.dma_start(out=wg_in.unsqueeze(0), in_=warm_sb)
        nc.gpsimd.collective_compute(
            "AllReduce", mybir.AluOpType.add,
            replica_groups=[list(range(NCORES))],
            ins=[w_in.opt()], outs=[w_out.opt()])
        nc.gpsimd.collective_compute(
            "AllGather", mybir.AluOpType.bypass,
            replica_groups=[list(range(NCORES))],
            ins=[wg_in.opt()], outs=[wg_out.opt()])

        # ---- loads, spread across engine queues so the QKV-critical
        # tensors (wq, x, wk, wv) land ASAP; FFN weights queue behind ----
        xs = []
        for img in range(BL):
            xs.append(data.tile([128, NCH, N], F16, name=f"xs{img}",
                                tag=f"xs{img}"))
        wq_sb = const.tile([128, NCH, C], F16, name="wq_sb")
        wk_sb = const.tile([128, NCH, C], F16, name="wk_sb")
        wv_sb = const.tile([128, NCH, NH * DH], F16, name="wv_sb")
        w1_sb = const.tile([128, NCH, DFF], F16, name="w1_sb")
        w2_sb = const.tile([128, NFC, C], F16, name="w2_sb")
        sm_sb = const.tile([128, 14], F32, name="sm_sb")

        x_r = x_d.ap().rearrange("b p (c n) -> b p c n", n=N)
        # x0 is split four ways across the three DMA queues so the first Q
        # matmul's inputs land ~3us sooner
        nc.scalar.dma_start(out=wq_sb,
                            in_=wq_d.ap().rearrange("p (k m) -> p k m", m=C))
        nc.scalar.dma_start(out=xs[0][:, 0, 512:], in_=x_r[0, :, 0, 512:])
        nc.scalar.dma_start(out=sm_sb, in_=sm_d.ap())
        nc.scalar.dma_start(out=xs[1][:, 1, :], in_=x_r[1, :, 1, :])
        nc.scalar.dma_start(out=w1_sb,
                            in_=w1_d.ap().rearrange("p (k m) -> p k m", m=DFF))
        nc.scalar.dma_start(out=w2_sb,
                            in_=w2_d.ap().rearrange("p (k m) -> p k m", m=C))
        nc.gpsimd.dma_start(out=xs[0][:, 0, 0:512], in_=x_r[0, :, 0, 0:512])
        nc.gpsimd.dma_start(out=xs[0][:, 1, 512:], in_=x_r[0, :, 1, 512:])
        nc.gpsimd.dma_start(out=wv_sb,
                            in_=wv_d.ap().rearrange("p (k m) -> p k m",
                                                    m=NH * DH))
        nc.sync.dma_start(out=wk_sb,
                          in_=wk_d.ap().rearrange("p (k m) -> p k m", m=C))
        nc.sync.dma_start(out=xs[0][:, 1, 0:512], in_=x_r[0, :, 1, 0:512])
        nc.sync.dma_start(out=xs[1][:, 0, :], in_=x_r[1, :, 0, :])

        bq_sb = sm_sb[:, 0:NCH]
        b1_sb = sm_sb[:, NCH:NCH + NFC]
        gam_sb = sm_sb[:, NCH + NFC:NCH + NFC + NCH]
        bet_sb = sm_sb[:, NCH + NFC + NCH:NCH + NFC + 2 * NCH]

        # PE warm-up: ~60 tiny matmuls straight after the preamble keep the
        # HAM activity window busy so QKV starts at 2.4GHz instead of 1.2.
        wrm_t = const.tile([128, 128], F16, name="wrm_t")
        nc.vector.memset(wrm_t, 0.5)
        wsink = const.tile([1, 4], F32, name="wsink")
        nc.vector.memset(wsink, 0.0)
        with tc.tile_pool(name="wrps", bufs=1, space="PSUM") as wrps:
            wp_t = wrps.tile([128, 128], F32, name="wp_t")
            for _ in range(72):
                nc.tensor.matmul(wp_t, lhsT=wrm_t, rhs=wrm_t,
                                 start=True, stop=True)
            # keep the dummies alive: route one lane to a DMA'd sink
            nc.vector.tensor_copy(wsink[0:1, 0:1], wp_t[0:1, 0:1])
        nc.scalar.dma_start(out=snk_d.ap().unsqueeze(0), in_=wsink)

        # ---- persistent SBUF tensors ----
        q_sb, k_sb, vt_sb, o_sb, mh_sb, u_sb = [], [], [], [], [], []
        for img in range(BL):
            q_sb.append(data.tile([128, NCH, N], F16, name=f"q{img}", tag=f"q{img}"))
            k_sb.append(data.tile([128, NCH, N], F16, name=f"k{img}", tag=f"k{img}"))
            vt_sb.append(data.tile([128, NNC, NH * 128], BF16, name=f"vt{img}",
                                   tag=f"vt{img}"))
            o_sb.append(data.tile([128, NCH, N], F32, name=f"o{img}", tag=f"o{img}"))
            mh_sb.append(data.tile([128, NCH, N], F16, name=f"mh{img}",
                                   tag=f"mh{img}"))
            u_sb.append(data.tile([128, NCH, N], F32, name=f"u{img}", tag=f"u{img}"))

        # V^T layout per head block (128 cols): even heads [v(64) | 1 | 0*63],
        # odd heads [1 | 0*63 | v(64)] -- the ones (denominator) column must
        # land on a 32-aligned PSUM partition (0 or 64).
        for img in range(BL):
            vt4 = vt_sb[img].rearrange("p a (h d) -> p a h d", d=128)
            for h in range(NH):
                if h % 2 == 0:
                    nc.gpsimd.memset(vt4[:, :, h, DH + 1:128], 0.0)
                    nc.gpsimd.memset(vt4[:, :, h, DH:DH + 1], 1.0)
                else:
                    nc.gpsimd.memset(vt4[:, :, h, 1:DH], 0.0)
                    nc.gpsimd.memset(vt4[:, :, h, 0:1], 1.0)

        st1 = work.tile([128, NCH, BL * 2, 6], F32, name="bn1_stats",
                        tag="bn1_stats")
        st2 = work.tile([128, NCH, BL * 2, 6], F32, name="bn2_stats",
                        tag="bn2_stats")

        # collective state: bn1 has one collective per channel chunk (the
        # first fires mid-attention, fully hidden); bn2 uses a single
        # collective for both chunks (two in the tail would serialize on
        # the one CC stream, ~11us each).  The latency-exposed collectives
        # (bn1_1, bn2) use AllGather + local sum: measured 7.3us CC wire
        # vs AllReduce's 10.8us for this payload (probe_cc.py).
        cc = {}
        for name, nch, gather in (("bn1_0", 1, False), ("bn1_1", 1, True),
                                  ("bn2_0", 1, True), ("bn2_1", 1, True)):
            mult = NCORES if gather else 1
            cin = dram.tile([128 * 2 * nch], F32, name=f"{name}_in",
                            tag=f"{name}_in")
            cout = dram.tile([128 * 2 * nch * mult], F32, name=f"{name}_out",
                             tag=f"{name}_out", addr_space="Shared")
            cc[name] = (cin, cout, gather)
        pk_t = work.tile([128, 2 * NCH, 2], F32, name="pk", tag="pk")
        mv_t = work.tile([128, 2 * NCH, 2], F32, name="mv", tag="mv")
        eps_sb = const.tile([128, 1], F32, name="eps_sb")
        nc.vector.memset(eps_sb, EPS)

        def bn_pack(name, ch, stats, slot, nch=1, cslot=0, eng=None):
            """Aggregate one chunk's stats, pack [mean, E[x^2]], ship to the
            collective input buffer (slot cslot of nch).  bn2's packs ride
            the gpsimd queue (same queue as the collective doorbell)."""
            nc.vector.bn_aggr(out=mv_t[:, slot, :], in_=stats[:, ch, :, :])
            nc.vector.tensor_mul(pk_t[:, slot, 0:1], mv_t[:, slot, 0:1],
                                 mv_t[:, slot, 0:1])
            nc.vector.tensor_add(pk_t[:, slot, 1:2], mv_t[:, slot, 1:2],
                                 pk_t[:, slot, 0:1])
            nc.vector.tensor_copy(pk_t[:, slot, 0:1], mv_t[:, slot, 0:1])
            cin = cc[name][0]
            ccr = cin.rearrange("(p k) -> p k", p=128)
            (eng or nc.sync).dma_start(out=ccr[:, 2 * cslot:2 * cslot + 2],
                                       in_=pk_t[:, slot, :])

        def bn_trigger(name):
            cin, cout, gather = cc[name]
            if gather:
                nc.gpsimd.collective_compute(
                    "AllGather", mybir.AluOpType.bypass,
                    replica_groups=[list(range(NCORES))],
                    ins=[cin.opt()], outs=[cout.opt()])
            else:
                nc.gpsimd.collective_compute(
                    "AllReduce", mybir.AluOpType.add,
                    replica_groups=[list(range(NCORES))],
                    ins=[cin.opt()], outs=[cout.opt()])

        def bn_finish(name, chs, scale_sb, shift_sb):
            """Reduced [sum(mean), sum(E[x^2])] -> scale/shift for the
            chunk list `chs`.  Short chain: 5 DVE ops + ACT sqrt + DVE
            reciprocal.  For AllGather collectives the 8 per-core blocks
            are loaded on 3 DMA queues and tree-summed on DVE first."""
            cout, gather = cc[name][1], cc[name][2]
            w = len(chs)
            sg = work.tile([128, w, 2], F32, name=f"{name}sg",
                           tag=f"{name}sg")
            if gather:
                gt = work.tile([128, NCORES, w, 2], F32, name=f"{name}gt",
                               tag=f"{name}gt")
                agr = cout.rearrange("(g p k) -> g p k", g=NCORES, p=128)
                for g in range(NCORES):
                    eng = (nc.sync, nc.scalar, nc.gpsimd)[g % 3]
                    eng.dma_start(out=gt[:, g], in_=agr[g])
                nc.vector.tensor_add(gt[:, 0:4], gt[:, 0:4], gt[:, 4:8])
                nc.vector.tensor_add(gt[:, 0:2], gt[:, 0:2], gt[:, 2:4])
                nc.vector.tensor_add(sg, gt[:, 0], gt[:, 1])
            else:
                nc.sync.dma_start(out=sg,
                                  in_=cout.rearrange("(p k) -> p k", p=128))
            g8 = work.tile([128, w, 2], F32, name=f"{name}g8",
                           tag=f"{name}g8")
            nc.vector.tensor_scalar_mul(g8, sg, 1.0 / NCORES)
            var_t = work.tile([128, w], F32, name=f"{name}v",
                              tag=f"{name}v")
            nc.vector.tensor_mul(var_t, g8[:, :, 0], g8[:, :, 0])
            nc.vector.tensor_sub(var_t, g8[:, :, 1], var_t)
            rs = work.tile([128, w], F32, name=f"{name}r", tag=f"{name}r")
            # rs <- sqrt(var + eps) on ACT, then 1/rs on DVE
            nc.scalar.activation(rs, var_t,
                                 mybir.ActivationFunctionType.Sqrt,
                                 bias=eps_sb[:, 0:1])
            nc.vector.reciprocal(rs, rs)
            tmp = work.tile([128, w], F32, name=f"{name}t", tag=f"{name}t")
            c0, cw = chs[0], chs[0] + w
            nc.vector.tensor_mul(scale_sb[:, c0:cw], gam_sb[:, c0:cw], rs)
            nc.vector.tensor_mul(tmp, g8[:, :, 0], scale_sb[:, c0:cw])
            nc.vector.tensor_sub(shift_sb[:, c0:cw], bet_sb[:, c0:cw], tmp)

        # =========== QKV (own pools) then heads (own pools) ====
        def make_qkv(qkps, vtps):
            def emit_qkv_q(img):
                for mc in range(NCH):
                    qp = qkps.tile([128, N], F32, tag="qp", bufs=2)
                    for kc in range(NCH):
                        for mv in range(2):
                            nc.tensor.matmul(
                                qp[:, mv * 512:(mv + 1) * 512],
                                lhsT=(wq_sb[:, kc, mc * 128:(mc + 1) * 128]),
                                rhs=(xs[img][:, kc, mv * 512:(mv + 1) * 512]),
                                start=(kc == 0), stop=(kc == NCH - 1))
                    nc.scalar.activation(q_sb[img][:, mc, :], qp,
                                         mybir.ActivationFunctionType.Identity,
                                         bias=bq_sb[:, mc:mc + 1])

            def emit_qkv_k(img):
                for mc in range(NCH):
                    kp = qkps.tile([128, N], F32, tag="qp", bufs=2)
                    for kc in range(NCH):
                        for mv in range(2):
                            nc.tensor.matmul(
                                kp[:, mv * 512:(mv + 1) * 512],
                                lhsT=(wk_sb[:, kc, mc * 128:(mc + 1) * 128]),
                                rhs=(xs[img][:, kc, mv * 512:(mv + 1) * 512]),
                                start=(kc == 0), stop=(kc == NCH - 1))
                    # plain copy: DVE, keeping ACT free for the first exps
                    nc.vector.tensor_copy(k_sb[img][:, mc, :], kp)

            def emit_qkv_v(img, pcs):
                vt4 = vt_sb[img].rearrange("p a (h d) -> p a h d", d=128)
                for pc in pcs:
                    vp = vtps.tile([128, N], F32, tag="vp", bufs=2)
                    for kc in range(NCH):
                        nc.tensor.matmul(
                            vp[:, 0:NH * DH],
                            lhsT=(xs[img][:, kc, pc * 128:(pc + 1) * 128]),
                            rhs=(wv_sb[:, kc, :]),
                            start=(kc == 0), stop=(kc == NCH - 1))
                    for h in range(NH):
                        dst0 = 0 if h % 2 == 0 else 64
                        nc.vector.tensor_copy(
                            vt4[:, pc, h, dst0:dst0 + DH],
                            vp[:, h * DH:(h + 1) * DH])

            return emit_qkv_q, emit_qkv_k, emit_qkv_v

        def make_head(etps, oaps):
            def emit_head(img, h, tail=False, mid=None):
                hc = h // 2
                oaug = oaps.tile([128, N], F32, tag="oaug", bufs=2,
                                 name=f"oaug{img}{h}")
                for pc in range(NNC):
                    et = etps.tile([128, N], F32, tag="et", bufs=2,
                                   name=f"et{pc}")
                    for mv in range(2):
                        nc.tensor.matmul(
                            et[:, mv * 512:(mv + 1) * 512],
                            lhsT=(k_sb[img][(h % 2) * 64:(h % 2) * 64 + 64,
                                            hc, pc * 128:(pc + 1) * 128]),
                            rhs=(q_sb[img][(h % 2) * 64:(h % 2) * 64 + 64,
                                           hc, mv * 512:(mv + 1) * 512]),
                            start=True, stop=True)
                    p_t = work.tile([128, N], BF16, name="p_t", tag="p_t",
                                    bufs=6)
                    nc.scalar.activation(p_t, et,
                                         mybir.ActivationFunctionType.Exp)
                    for mv in range(2):
                        nc.tensor.matmul(
                            oaug[:, mv * 512:(mv + 1) * 512],
                            lhsT=(vt_sb[img][:, pc, h * 128:(h + 1) * 128]),
                            rhs=(p_t[:, mv * 512:(mv + 1) * 512]),
                            start=(pc == 0), stop=(pc == NNC - 1))
                den_chain(img, h, oaug, tail=tail, mid=mid)

            def den_chain(img, h, oaug, tail=False, mid=None):
                hc, ho = h // 2, (h % 2) * 64
                denp = 64 if h % 2 == 0 else 0
                # softmax denominator: row `denp` of oaug.  Reciprocal of
                # the single row first (saves a copy on the critical DVE
                # chain), broadcast across partitions on GPSIMD, then scale
                # the head's 64 rows.  For the chunk-final heads the chain
                # is split into halves so the BN1 stats (and the AllReduce
                # behind them) start sooner, with the residual add moved to
                # GPSIMD so the DVE chain stays short.
                halves = ((0, 512), (512, 1024)) if tail else ((0, 1024),)
                dsb = work.tile([128, N], F32, name="dsb", tag="dsb", bufs=2)
                dbc = work.tile([128, N], F32, name="dbc", tag="dbc", bufs=2)
                rbc = work.tile([128, N], F32, name="rbc", tag="rbc", bufs=2)
                dsb0 = None
                for lo, hi in halves:
                    if denp == 0:
                        # odd heads: denominator row already at partition 0,
                        # so reciprocal the single row first (the custom DVE
                        # op reads absolute partition 0) and broadcast the
                        # result -- one DVE op shorter on the critical chain
                        nc.vector.reciprocal_approx_fast(
                            out=dsb[0:1, lo:hi], in_=oaug[0:1, lo:hi])
                        nc.gpsimd.partition_broadcast(rbc[:, lo:hi],
                                                      dsb[0:1, lo:hi])
                        if mid is not None and lo > 0:
                            # inject independent work (BN1 chunk-0 finish)
                            # into the DVE slot that otherwise idles while
                            # GPSIMD broadcasts the second half
                            mid()
                            mid = None
                    else:
                        # even heads: both the custom reciprocal and
                        # partition_broadcast read ABSOLUTE partition 0 on
                        # HW -- copy the row out of PSUM, bounce it down
                        # with a small SBUF DMA, broadcast, then reciprocal
                        nc.vector.tensor_copy(dsb[denp:denp + 1, lo:hi],
                                              oaug[denp:denp + 1, lo:hi])
                        if dsb0 is None:
                            dsb0 = work.tile([128, N], F32, name="dsb0",
                                             tag="dsb0", bufs=2)
                        nc.gpsimd.dma_start(out=dsb0[0:1, lo:hi],
                                            in_=dsb[denp:denp + 1, lo:hi])
                        nc.gpsimd.partition_broadcast(dbc[:, lo:hi],
                                                      dsb0[0:1, lo:hi])
                        nc.vector.reciprocal_approx_fast(out=rbc[:, lo:hi],
                                                         in_=dbc[:, lo:hi])
                    nc.vector.tensor_mul(o_sb[img][ho:ho + 64, hc, lo:hi],
                                         oaug[ho:ho + 64, lo:hi],
                                         rbc[ho:ho + 64, lo:hi])
                    if h % 2 == 1 and tail:
                        sg = lo // 512
                        nc.gpsimd.tensor_add(
                            o_sb[img][:, hc, lo:hi],
                            o_sb[img][:, hc, lo:hi], xs[img][:, hc, lo:hi])
                        nc.vector.bn_stats(
                            out=st1[:, hc, img * 2 + sg, :],
                            in_=o_sb[img][:, hc, lo:hi])
                if h % 2 == 1 and not tail:
                    # both heads of chunk hc done -> residual + local stats
                    nc.vector.tensor_add(o_sb[img][:, hc, :],
                                         o_sb[img][:, hc, :],
                                         xs[img][:, hc, :])
                    for sg in range(2):
                        nc.vector.bn_stats(
                            out=st1[:, hc, img * 2 + sg, :],
                            in_=o_sb[img][:, hc, sg * 512:(sg + 1) * 512])

            return emit_head

        with tc.tile_pool(name="qkps", bufs=2, space="PSUM") as qkps, \
             tc.tile_pool(name="vtps", bufs=2, space="PSUM") as vtps:
            eq, ek, ev = make_qkv(qkps, vtps)
            for img in range(BL):
                eq(img)
                ek(img)
                ev(img, range(NNC))

        s1_sb = work.tile([128, NCH], F32, name="s1_sb", tag="bns")
        t1_sb = work.tile([128, NCH], F32, name="t1_sb", tag="bnt")
        with tc.tile_pool(name="etps", bufs=2, space="PSUM") as etps, \
             tc.tile_pool(name="oaps", bufs=2, space="PSUM") as oaps:
            eh = make_head(etps, oaps)
            # phase A: channel chunk 0 (heads 0,1) for both images, then
            # fire its stats AllReduce mid-attention (fully hidden)
            eh(0, 0)
            eh(0, 1)
            eh(1, 0)
            eh(1, 1)
            bn_pack("bn1_0", 0, st1, slot=0)
            bn_trigger("bn1_0")
            # phase B: chunk 1 (heads 2,3).  Chunk 0's finish + apply are
            # emitted only after the last head AND after chunk 1's
            # collective fires: its result has been sitting in DRAM since
            # mid-attention, and emitting the dependent ops earlier risks
            # blocking the ACT/DVE streams mid-attention on a slow run.
            def finish_apply_ch0():
                # chunk-0 scale/shift + BN1 apply.  Both images on ACT
                # (idle after the last exp) so the DVE tail chain is not
                # lengthened; its result has been in DRAM since
                # mid-attention, so no engine blocks here.
                bn_finish("bn1_0", [0], s1_sb, t1_sb)
                for img in range(BL):
                    nc.scalar.activation(
                        mh_sb[img][:, 0, :], o_sb[img][:, 0, :],
                        mybir.ActivationFunctionType.Identity,
                        bias=t1_sb[:, 0:1], scale=s1_sb[:, 0:1])

            eh(0, 2)
            eh(0, 3)
            eh(1, 2)
            eh(1, 3, tail=True, mid=finish_apply_ch0)
            bn_pack("bn1_1", 1, st1, slot=1)
            bn_trigger("bn1_1")

            # =========== FFN W1 (inside the attention pools: fp tiles
            # reuse the et/oaug buffer rotation so the first prestart
            # matmuls start the moment the last PV/exp frees a buffer,
            # instead of waiting for the whole pool to drain) ===========
            ffs = [[work.tile([128, N], F16, name=f"ffs{img}_{fc}",
                              tag=f"ffs{img}_{fc}") for fc in range(NFC)]
                   for img in range(BL)]

            fp_n = [0]

            def fp_alloc():
                fp_n[0] += 1
                if fp_n[0] % 2 == 1:
                    fp = etps.tile([128, N], F32, tag="et", bufs=2,
                                   name=f"fp{fp_n[0]}")
                else:
                    fp = oaps.tile([128, N], F32, tag="oaug", bufs=2,
                                   name=f"fp{fp_n[0]}")
                return fp

            # prestart: chunk-0 contribution for the first EIGHT (img, fc)
            # tiles while chunk 1's collective drains.  PSUM only holds 4,
            # so the first 4 close their accumulation group and spill to
            # f16 SBUF (ACT/DVE are idle during the stall); the next 4 stay
            # resident with their group open for the kc=1 continuation.
            spill = [(0, 0), (1, 0), (0, 1), (1, 1)]
            resid = [(0, 2), (1, 2), (0, 3), (1, 3)]
            sp_sb = {}
            fp_pre = {}
            for i, (img, fc) in enumerate(spill):
                fp = fp_alloc()
                for mv in range(2):
                    nc.tensor.matmul(
                        fp[:, mv * 512:(mv + 1) * 512],
                        lhsT=(w1_sb[:, 0, fc * 128:(fc + 1) * 128]),
                        rhs=(mh_sb[img][:, 0, mv * 512:(mv + 1) * 512]),
                        start=True, stop=True)
                sp = work.tile([128, N], F16, name=f"sp{i}", tag=f"sp{i}")
                if i % 2 == 0:
                    nc.scalar.activation(
                        sp, fp, mybir.ActivationFunctionType.Identity)
                else:
                    nc.vector.tensor_copy(sp, fp)
                sp_sb[(img, fc)] = sp
            for img, fc in resid:
                fp = fp_alloc()
                fp_pre[(img, fc)] = fp
                for mv in range(2):
                    nc.tensor.matmul(
                        fp[:, mv * 512:(mv + 1) * 512],
                        lhsT=(w1_sb[:, 0, fc * 128:(fc + 1) * 128]),
                        rhs=(mh_sb[img][:, 0, mv * 512:(mv + 1) * 512]),
                        start=True, stop=False)
            # chunk 1 scale/shift + apply (waits on its collective)
            bn_finish("bn1_1", [1], s1_sb, t1_sb)
            for img in range(BL):
                if img == 0:
                    nc.scalar.activation(
                        mh_sb[img][:, 1, :], o_sb[img][:, 1, :],
                        mybir.ActivationFunctionType.Identity,
                        bias=t1_sb[:, 1:2], scale=s1_sb[:, 1:2])
                else:
                    nc.vector.tensor_scalar(
                        out=mh_sb[img][:, 1, :], in0=o_sb[img][:, 1, :],
                        scalar1=s1_sb[:, 1:2], scalar2=t1_sb[:, 1:2],
                        op0=mybir.AluOpType.mult, op1=mybir.AluOpType.add)

            def finish_w1(img, fc, fp):
                for mv in range(2):
                    nc.tensor.matmul(
                        fp[:, mv * 512:(mv + 1) * 512],
                        lhsT=(w1_sb[:, 1, fc * 128:(fc + 1) * 128]),
                        rhs=(mh_sb[img][:, 1, mv * 512:(mv + 1) * 512]),
                        start=False, stop=True)
                nc.scalar.activation(
                    ffs[img][fc], fp,
                    mybir.ActivationFunctionType.Prelu,
                    bias=b1_sb[:, fc:fc + 1], alpha=a_slope)

            # residents first (their kc=1 continues the open group and
            # frees the PSUM buffers the spilled tiles' kc=1 will rotate
            # into), then the spilled four (fresh psum + add + prelu)
            for img, fc in resid:
                finish_w1(img, fc, fp_pre[(img, fc)])
            for img, fc in spill:
                fq = fp_alloc()
                for mv in range(2):
                    nc.tensor.matmul(
                        fq[:, mv * 512:(mv + 1) * 512],
                        lhsT=(w1_sb[:, 1, fc * 128:(fc + 1) * 128]),
                        rhs=(mh_sb[img][:, 1, mv * 512:(mv + 1) * 512]),
                        start=True, stop=True)
                ft = work.tile([128, N], F16, name=f"ft{img}{fc}",
                               tag="ftmp", bufs=2)
                nc.vector.tensor_add(ft, fq, sp_sb[(img, fc)])
                nc.scalar.activation(
                    ffs[img][fc], ft,
                    mybir.ActivationFunctionType.Prelu,
                    bias=b1_sb[:, fc:fc + 1], alpha=a_slope)
            for img in range(BL):
                for fc in range(4, NFC):
                    fp = fp_alloc()
                    for kc in range(NCH):
                        for mv in range(2):
                            nc.tensor.matmul(
                                fp[:, mv * 512:(mv + 1) * 512],
                                lhsT=(w1_sb[:, kc, fc * 128:(fc + 1) * 128]),
                                rhs=(mh_sb[img][:, kc,
                                                mv * 512:(mv + 1) * 512]),
                                start=(kc == 0), stop=(kc == NCH - 1))
                    nc.scalar.activation(
                        ffs[img][fc], fp,
                        mybir.ActivationFunctionType.Prelu,
                        bias=b1_sb[:, fc:fc + 1], alpha=a_slope)

        # W2, mc-major: chunk 0's stats collective fires after half the
        # work and hides under chunk 1's matmuls
        with tc.tile_pool(name="ops2", bufs=2, space="PSUM") as ops2:
            for mc in range(NCH):
                for img in range(BL):
                    outp = ops2.tile([128, N], F32, tag="outp", bufs=2)
                    for fc in range(NFC):
                        for mv in range(2):
                            nc.tensor.matmul(
                                outp[:, mv * 512:(mv + 1) * 512],
                                lhsT=(w2_sb[:, fc, mc * 128:(mc + 1) * 128]),
                                rhs=(ffs[img][fc][:, mv * 512:(mv + 1) * 512]),
                                start=(fc == 0), stop=(fc == NFC - 1))
                    nc.vector.tensor_add(u_sb[img][:, mc, :], outp,
                                         mh_sb[img][:, mc, :])
                    for sg in range(2):
                        nc.vector.bn_stats(
                            out=st2[:, mc, img * 2 + sg, :],
                            in_=u_sb[img][:, mc, sg * 512:(sg + 1) * 512])
                # fire each chunk's AllGather as soon as its stats exist:
                # chunk 0's collective, finish, apply and output DMA all
                # overlap chunk 1's matmuls and collective, so the tail
                # pays only one chunk's latency
                bn_pack(f"bn2_{mc}", mc, st2, slot=NCH + mc, eng=nc.gpsimd)
                bn_trigger(f"bn2_{mc}")

        # =========== BN2 + output (f16, upcast host-side) ===========
        s2_sb = work.tile([128, NCH], F32, name="s2_sb", tag="bns2")
        t2_sb = work.tile([128, NCH], F32, name="t2_sb", tag="bnt2")
        # apply: mh_sb is dead -> reuse as f16 staging; img0 on ACT,
        # img1 on DVE, in half-chunks so the output DMAs start early
        for mc in range(NCH):
            bn_finish(f"bn2_{mc}", [mc], s2_sb, t2_sb)
            for img in range(BL):
                outr = out_d.ap()[img].rearrange("(c p) n -> p c n", p=128)
                for mv in range(2):
                    sl = slice(mv * 512, (mv + 1) * 512)
                    if img == 0:
                        nc.scalar.activation(
                            mh_sb[img][:, mc, sl], u_sb[img][:, mc, sl],
                            mybir.ActivationFunctionType.Identity,
                            bias=t2_sb[:, mc:mc + 1], scale=s2_sb[:, mc:mc + 1])
                        nc.sync.dma_start(out=outr[:, mc, sl],
                                          in_=mh_sb[img][:, mc, sl])
                    else:
                        nc.vector.tensor_scalar(
                            out=mh_sb[img][:, mc, sl], in0=u_sb[img][:, mc, sl],
                            scalar1=s2_sb[:, mc:mc + 1],
                            scalar2=t2_sb[:, mc:mc + 1],
                            op0=mybir.AluOpType.mult, op1=mybir.AluOpType.add)
                        nc.gpsimd.dma_start(out=outr[:, mc, sl],
                                            in_=mh_sb[img][:, mc, sl])


_COMPILED = None


def _get_compiled(a_slope: float):
    global _COMPILED
    if _COMPILED is None or _COMPILED[0] != a_slope:
        _COMPILED = (a_slope, _build(a_slope))
    return _COMPILED[1]


def _prep_inputs(inputs):
    x = np.ascontiguousarray(np.asarray(inputs["x"], dtype=np.float32))
    Wq = np.asarray(inputs["Wq"], dtype=np.float32)
    Wk = np.asarray(inputs["Wk"], dtype=np.float32)
    Wv = np.asarray(inputs["Wv"], dtype=np.float32)
    bq = np.asarray(inputs["bq"], dtype=np.float32)
    W1 = np.asarray(inputs["W1"], dtype=np.float32)
    b1 = np.asarray(inputs["b1"], dtype=np.float32)
    W2 = np.asarray(inputs["W2"], dtype=np.float32)
    gamma = np.asarray(inputs["gamma"], dtype=np.float32)
    beta = np.asarray(inputs["beta"], dtype=np.float32)

    def pack(mat):
        # [K, M] (K = contraction, rows grouped as (chunk, partition)) ->
        # [128, n_chunks * M] partition-major so the DMA is 128 big rows
        K, M = mat.shape
        return np.ascontiguousarray(
            mat.reshape(K // 128, 128, M).transpose(1, 0, 2).reshape(128, -1)
            .astype(np.float16))

    wvT = np.zeros((C, NH * DH), dtype=np.float32)
    for h in range(NH):
        wvT[:, h * DH:(h + 1) * DH] = Wv[h].T
    sm = np.zeros((128, 14), dtype=np.float32)
    sm[:, 0:NCH] = bq.reshape(NCH, 128).T
    sm[:, NCH:NCH + NFC] = b1.reshape(NFC, 128).T
    sm[:, NCH + NFC:NCH + NFC + NCH] = gamma.reshape(NCH, 128).T
    sm[:, NCH + NFC + NCH:] = beta.reshape(NCH, 128).T
    common = {
        "wqT": pack(Wq.reshape(C, C).T),
        "wkT": pack(Wk.reshape(C, C).T),
        "wvT": pack(wvT),
        "w1T": pack(W1.T),
        "w2T": pack(W2.T),
        "sm": sm,
    }
    # x: [B, C, N] -> per-core [BL, 128, NCH*N] partition-major f16
    xp = np.ascontiguousarray(
        x.reshape(B, NCH, 128, N).transpose(0, 2, 1, 3).reshape(B, 128, NCH * N)
        .astype(np.float16))
    in_maps = []
    for c in range(NCORES):
        m = dict(common)
        m["x"] = np.ascontiguousarray(xp[c * BL:(c + 1) * BL])
        in_maps.append(m)
    return in_maps


def kernel_ex(trace=False, **inputs):
    a_slope = float(np.asarray(inputs["a"]))
    nc = _get_compiled(a_slope)
    in_maps = _prep_inputs(inputs)
    res = bass_utils.run_bass_kernel_spmd(
        nc, in_maps, core_ids=list(range(NCORES)), trace=trace)
    out = np.empty((B, C, N), dtype=np.float32)
    for c in range(NCORES):
        out[c * BL:(c + 1) * BL] = res.results[c]["out"].astype(np.float32)
    return out.reshape(B, C, HH, WW), res


def kernel(**inputs):
    out, _ = kernel_ex(False, **inputs)
    return out


# revision 42
# speedup vs baseline: 1.0922x; 1.0922x over previous
"""Trainium2 Bass kernel for MultiHeadPosAttn (attention + BN + FFN + BN).

Sharding: data-parallel over batch across 8 NeuronCores (2 images/core).
BatchNorm batch statistics are combined with tiny (1KB) AllReduces.

Math notes (verified exactly equivalent to the reference):
  - bk cancels in softmax (adds a per-query constant to every logit row).
  - bv cancels in BN1 (per-channel constant shift; softmax rows sum to 1).
  - b2 cancels in BN2 (per-channel constant shift).
  - PReLU(y) = Lrelu(y) with alpha = a (ACT supports a slope parameter).
  - softmax needs no max-subtraction: |logits| <= ~66 so exp() stays in
    fp32 range (max ~3e28 << 3.4e38).
Softmax denominator comes from an extra all-ones column in each head's
V^T block, so the attention matmul also produces sum_k(P) per query.

Collective scheduling (the previous version lost ~86us to two exposed
AllReduce waits): each BN's stats are now reduced with TWO collectives,
one per 128-channel chunk.  The first fires at the halfway point of the
producing phase (mid-attention for BN1, after W2 chunk 0 for BN2) so its
latency hides entirely under compute; the second fires at the end and is
covered by (a) the other chunk's scale/shift finish + apply, (b) partial
W1 accumulation over the already-available channel chunk (BN1), or the
other chunk's apply + output DMA (BN2).  Output is written f16 and
upcast host-side (rel tolerance is 2e-2; f16 rounds at 5e-4).
"""

import numpy as np

import concourse.bass as bass
import concourse.bacc as bacc
import concourse.tile as tile
from concourse import mybir
from concourse import bass_utils

F32 = mybir.dt.float32
BF16 = mybir.dt.bfloat16
F16 = mybir.dt.float16

B, C, HH, WW = 16, 256, 32, 32
N = HH * WW              # 1024 spatial positions
NH, DH = 4, 64           # heads, head dim
DFF = 4 * C              # 1024
EPS = 1e-5
NCORES = 8
BL = B // NCORES         # 2 images per core
NCH = C // 128           # 2 channel chunks of 128
NFC = DFF // 128         # 8 ffn chunks
NNC = N // 128           # 8 position chunks


def _build(a_slope: float):
    nc = bacc.Bacc("TRN2", target_bir_lowering=False, debug=False,
                   num_devices=NCORES)

    # all big inputs are pre-packed host-side into [128, free] partition-major
    # layouts so every load DMA is 128 descriptors of >=512B contiguous rows
    x_d = nc.dram_tensor("x", [BL, 128, NCH * N], F16, kind="ExternalInput")
    wq_d = nc.dram_tensor("wqT", [128, NCH * C], F16, kind="ExternalInput")
    wk_d = nc.dram_tensor("wkT", [128, NCH * C], F16, kind="ExternalInput")
    wv_d = nc.dram_tensor("wvT", [128, NCH * NH * DH], F16, kind="ExternalInput")
    w1_d = nc.dram_tensor("w1T", [128, NCH * DFF], F16, kind="ExternalInput")
    w2_d = nc.dram_tensor("w2T", [128, NFC * C], F16, kind="ExternalInput")
    # bq(2) | b1(8) | gamma(2) | beta(2) packed per partition
    sm_d = nc.dram_tensor("sm", [128, 14], F32, kind="ExternalInput")
    out_d = nc.dram_tensor("out", [BL, C, N], F16, kind="ExternalOutput")
    snk_d = nc.dram_tensor("snk", [4], F32, kind="ExternalOutput")

    with tile.TileContext(nc) as tc:
        _emit(tc, a_slope,
              x_d=x_d, wq_d=wq_d, wk_d=wk_d, wv_d=wv_d,
              w1_d=w1_d, w2_d=w2_d, sm_d=sm_d, out_d=out_d, snk_d=snk_d)
    nc.compile()
    return nc


def _emit(tc, a_slope, *, x_d, wq_d, wk_d, wv_d, w1_d, w2_d, sm_d, out_d,
          snk_d):
    nc = tc.nc
    from contextlib import ExitStack

    ctx = ExitStack()
    with ctx:
        const = ctx.enter_context(tc.tile_pool(name="const", bufs=1))
        data = ctx.enter_context(tc.tile_pool(name="data", bufs=1))
        work = ctx.enter_context(tc.tile_pool(name="work", bufs=1))
        dram = ctx.enter_context(tc.tile_pool(name="dram", bufs=1, space="DRAM"))

        # ---- warm-up collectives fire FIRST so the one-time CC channel
        # setup (and any cross-core launch skew) drains before the real
        # BN collectives need the stream.  Both the AllReduce AND the
        # AllGather paths are warmed (the runtime sets their channels up
        # separately; bn1_1/bn2 use AllGather) ----
        warm_sb = const.tile([1, 64], F32, name="warm_sb")
        nc.vector.memset(warm_sb, 0.0)
        w_in = dram.tile([64], F32, name="warm_in", tag="warm_in")
        w_out = dram.tile([64], F32, name="warm_out", tag="warm_out",
                          addr_space="Shared")
        wg_in = dram.tile([64], F32, name="warmg_in", tag="warmg_in")
        wg_out = dram.tile([64 * NCORES], F32, name="warmg_out",
                           tag="warmg_out", addr_space="Shared")
        nc.sync.dma_start(out=w_in.unsqueeze(0), in_=warm_sb)
        nc.sync.dma_start(out=wg_in.unsqueeze(0), in_=warm_sb)
        nc.gpsimd.collective_compute(
            "AllReduce", mybir.AluOpType.add,
            replica_groups=[list(range(NCORES))],
            ins=[w_in.opt()], outs=[w_out.opt()])
        nc.gpsimd.collective_compute(
            "AllGather", mybir.AluOpType.bypass,
            replica_groups=[list(range(NCORES))],
            ins=[wg_in.opt()], outs=[wg_out.opt()])

        # ---- loads, spread across engine queues so the QKV-critical
        # tensors (wq, x, wk, wv) land ASAP; FFN weights queue behind ----
        xs = []
        for img in range(BL):
            xs.append(data.tile([128, NCH, N], F16, name=f"xs{img}",
                                tag=f"xs{img}"))
        wq_sb = const.tile([128, NCH, C], F16, name="wq_sb")
        wk_sb = const.tile([128, NCH, C], F16, name="wk_sb")
        wv_sb = const.tile([128, NCH, NH * DH], F16, name="wv_sb")
        w1_sb = const.tile([128, NCH, DFF], F16, name="w1_sb")
        w2_sb = const.tile([128, NFC, C], F16, name="w2_sb")
        sm_sb = const.tile([128, 14], F32, name="sm_sb")

        x_r = x_d.ap().rearrange("b p (c n) -> b p c n", n=N)
        # x0 is split four ways across the three DMA queues so the first Q
        # matmul's inputs land ~3us sooner
        nc.scalar.dma_start(out=wq_sb,
                            in_=wq_d.ap().rearrange("p (k m) -> p k m", m=C))
        nc.scalar.dma_start(out=xs[0][:, 0, 512:], in_=x_r[0, :, 0, 512:])
        nc.scalar.dma_start(out=sm_sb, in_=sm_d.ap())
        nc.scalar.dma_start(out=xs[1][:, 1, :], in_=x_r[1, :, 1, :])
        nc.scalar.dma_start(out=w1_sb,
                            in_=w1_d.ap().rearrange("p (k m) -> p k m", m=DFF))
        nc.scalar.dma_start(out=w2_sb,
                            in_=w2_d.ap().rearrange("p (k m) -> p k m", m=C))
        nc.gpsimd.dma_start(out=xs[0][:, 0, 0:512], in_=x_r[0, :, 0, 0:512])
        nc.gpsimd.dma_start(out=xs[0][:, 1, 512:], in_=x_r[0, :, 1, 512:])
        nc.gpsimd.dma_start(out=wv_sb,
                            in_=wv_d.ap().rearrange("p (k m) -> p k m",
                                                    m=NH * DH))
        nc.sync.dma_start(out=wk_sb,
                          in_=wk_d.ap().rearrange("p (k m) -> p k m", m=C))
        nc.sync.dma_start(out=xs[0][:, 1, 0:512], in_=x_r[0, :, 1, 0:512])
        nc.sync.dma_start(out=xs[1][:, 0, :], in_=x_r[1, :, 0, :])

        bq_sb = sm_sb[:, 0:NCH]
        b1_sb = sm_sb[:, NCH:NCH + NFC]
        gam_sb = sm_sb[:, NCH + NFC:NCH + NFC + NCH]
        bet_sb = sm_sb[:, NCH + NFC + NCH:NCH + NFC + 2 * NCH]

        # PE warm-up: ~60 tiny matmuls straight after the preamble keep the
        # HAM activity window busy so QKV starts at 2.4GHz instead of 1.2.
        wrm_t = const.tile([128, 128], F16, name="wrm_t")
        nc.vector.memset(wrm_t, 0.5)
        wsink = const.tile([1, 4], F32, name="wsink")
        nc.vector.memset(wsink, 0.0)
        with tc.tile_pool(name="wrps", bufs=1, space="PSUM") as wrps:
            wp_t = wrps.tile([128, 128], F32, name="wp_t")
            for _ in range(72):
                nc.tensor.matmul(wp_t, lhsT=wrm_t, rhs=wrm_t,
                                 start=True, stop=True)
            # keep the dummies alive: route one lane to a DMA'd sink
            nc.vector.tensor_copy(wsink[0:1, 0:1], wp_t[0:1, 0:1])
        nc.scalar.dma_start(out=snk_d.ap().unsqueeze(0), in_=wsink)

        # ---- persistent SBUF tensors ----
        q_sb, k_sb, vt_sb, o_sb, mh_sb, u_sb = [], [], [], [], [], []
        for img in range(BL):
            q_sb.append(data.tile([128, NCH, N], F16, name=f"q{img}", tag=f"q{img}"))
            k_sb.append(data.tile([128, NCH, N], F16, name=f"k{img}", tag=f"k{img}"))
            vt_sb.append(data.tile([128, NNC, NH * 128], BF16, name=f"vt{img}",
                                   tag=f"vt{img}"))
            o_sb.append(data.tile([128, NCH, N], F32, name=f"o{img}", tag=f"o{img}"))
            mh_sb.append(data.tile([128, NCH, N], F16, name=f"mh{img}",
                                   tag=f"mh{img}"))
            u_sb.append(data.tile([128, NCH, N], F32, name=f"u{img}", tag=f"u{img}"))

        # V^T layout per head block (128 cols): even heads [v(64) | 1 | 0*63],
        # odd heads [1 | 0*63 | v(64)] -- the ones (denominator) column must
        # land on a 32-aligned PSUM partition (0 or 64).
        for img in range(BL):
            vt4 = vt_sb[img].rearrange("p a (h d) -> p a h d", d=128)
            for h in range(NH):
                if h % 2 == 0:
                    nc.gpsimd.memset(vt4[:, :, h, DH + 1:128], 0.0)
                    nc.gpsimd.memset(vt4[:, :, h, DH:DH + 1], 1.0)
                else:
                    nc.gpsimd.memset(vt4[:, :, h, 1:DH], 0.0)
                    nc.gpsimd.memset(vt4[:, :, h, 0:1], 1.0)

        st1 = work.tile([128, NCH, BL * 2, 6], F32, name="bn1_stats",
                        tag="bn1_stats")
        st2 = work.tile([128, NCH, BL * 2, 6], F32, name="bn2_stats",
                        tag="bn2_stats")

        # collective state: bn1 has one collective per channel chunk (the
        # first fires mid-attention, fully hidden); bn2 uses a single
        # collective for both chunks (two in the tail would serialize on
        # the one CC stream, ~11us each).  The latency-exposed collectives
        # (bn1_1, bn2) use AllGather + local sum: measured 7.3us CC wire
        # vs AllReduce's 10.8us for this payload (probe_cc.py).
        cc = {}
        for name, nch, gather in (("bn1_0", 1, False), ("bn1_1", 1, True),
                                  ("bn2", NCH, True)):
            mult = NCORES if gather else 1
            cin = dram.tile([128 * 2 * nch], F32, name=f"{name}_in",
                            tag=f"{name}_in")
            cout = dram.tile([128 * 2 * nch * mult], F32, name=f"{name}_out",
                             tag=f"{name}_out", addr_space="Shared")
            cc[name] = (cin, cout, gather)
        pk_t = work.tile([128, 2 * NCH, 2], F32, name="pk", tag="pk")
        mv_t = work.tile([128, 2 * NCH, 2], F32, name="mv", tag="mv")
        eps_sb = const.tile([128, 1], F32, name="eps_sb")
        nc.vector.memset(eps_sb, EPS)

        def bn_pack(name, ch, stats, slot, nch=1, cslot=0, eng=None):
            """Aggregate one chunk's stats, pack [mean, E[x^2]], ship to the
            collective input buffer (slot cslot of nch).  bn2's packs ride
            the gpsimd queue (same queue as the collective doorbell)."""
            nc.vector.bn_aggr(out=mv_t[:, slot, :], in_=stats[:, ch, :, :])
            nc.vector.tensor_mul(pk_t[:, slot, 0:1], mv_t[:, slot, 0:1],
                                 mv_t[:, slot, 0:1])
            nc.vector.tensor_add(pk_t[:, slot, 1:2], mv_t[:, slot, 1:2],
                                 pk_t[:, slot, 0:1])
            nc.vector.tensor_copy(pk_t[:, slot, 0:1], mv_t[:, slot, 0:1])
            cin = cc[name][0]
            ccr = cin.rearrange("(p k) -> p k", p=128)
            (eng or nc.sync).dma_start(out=ccr[:, 2 * cslot:2 * cslot + 2],
                                       in_=pk_t[:, slot, :])

        def bn_trigger(name):
            cin, cout, gather = cc[name]
            if gather:
                nc.gpsimd.collective_compute(
                    "AllGather", mybir.AluOpType.bypass,
                    replica_groups=[list(range(NCORES))],
                    ins=[cin.opt()], outs=[cout.opt()])
            else:
                nc.gpsimd.collective_compute(
                    "AllReduce", mybir.AluOpType.add,
                    replica_groups=[list(range(NCORES))],
                    ins=[cin.opt()], outs=[cout.opt()])

        def bn_finish(name, chs, scale_sb, shift_sb):
            """Reduced [sum(mean), sum(E[x^2])] -> scale/shift for the
            chunk list `chs`.  Short chain: 5 DVE ops + ACT sqrt + DVE
            reciprocal.  For AllGather collectives the 8 per-core blocks
            are loaded on 3 DMA queues and tree-summed on DVE first."""
            cout, gather = cc[name][1], cc[name][2]
            w = len(chs)
            sg = work.tile([128, w, 2], F32, name=f"{name}sg",
                           tag=f"{name}sg")
            if gather:
                gt = work.tile([128, NCORES, w, 2], F32, name=f"{name}gt",
                               tag=f"{name}gt")
                agr = cout.rearrange("(g p k) -> g p k", g=NCORES, p=128)
                for g in range(NCORES):
                    eng = (nc.sync, nc.scalar, nc.gpsimd)[g % 3]
                    eng.dma_start(out=gt[:, g], in_=agr[g])
                nc.vector.tensor_add(gt[:, 0:4], gt[:, 0:4], gt[:, 4:8])
                nc.vector.tensor_add(gt[:, 0:2], gt[:, 0:2], gt[:, 2:4])
                nc.vector.tensor_add(sg, gt[:, 0], gt[:, 1])
            else:
                nc.sync.dma_start(out=sg,
                                  in_=cout.rearrange("(p k) -> p k", p=128))
            g8 = work.tile([128, w, 2], F32, name=f"{name}g8",
                           tag=f"{name}g8")
            nc.vector.tensor_scalar_mul(g8, sg, 1.0 / NCORES)
            var_t = work.tile([128, w], F32, name=f"{name}v",
                              tag=f"{name}v")
            nc.vector.tensor_mul(var_t, g8[:, :, 0], g8[:, :, 0])
            nc.vector.tensor_sub(var_t, g8[:, :, 1], var_t)
            rs = work.tile([128, w], F32, name=f"{name}r", tag=f"{name}r")
            # rs <- sqrt(var + eps) on ACT, then 1/rs on DVE
            nc.scalar.activation(rs, var_t,
                                 mybir.ActivationFunctionType.Sqrt,
                                 bias=eps_sb[:, 0:1])
            nc.vector.reciprocal(rs, rs)
            tmp = work.tile([128, w], F32, name=f"{name}t", tag=f"{name}t")
            c0, cw = chs[0], chs[0] + w
            nc.vector.tensor_mul(scale_sb[:, c0:cw], gam_sb[:, c0:cw], rs)
            nc.vector.tensor_mul(tmp, g8[:, :, 0], scale_sb[:, c0:cw])
            nc.vector.tensor_sub(shift_sb[:, c0:cw], bet_sb[:, c0:cw], tmp)

        # =========== QKV (own pools) then heads (own pools) ====
        def make_qkv(qkps, vtps):
            def emit_qkv_q(img):
                for mc in range(NCH):
                    qp = qkps.tile([128, N], F32, tag="qp", bufs=2)
                    for kc in range(NCH):
                        for mv in range(2):
                            nc.tensor.matmul(
                                qp[:, mv * 512:(mv + 1) * 512],
                                lhsT=(wq_sb[:, kc, mc * 128:(mc + 1) * 128]),
                                rhs=(xs[img][:, kc, mv * 512:(mv + 1) * 512]),
                                start=(kc == 0), stop=(kc == NCH - 1))
                    nc.scalar.activation(q_sb[img][:, mc, :], qp,
                                         mybir.ActivationFunctionType.Identity,
                                         bias=bq_sb[:, mc:mc + 1])

            def emit_qkv_k(img):
                for mc in range(NCH):
                    kp = qkps.tile([128, N], F32, tag="qp", bufs=2)
                    for kc in range(NCH):
                        for mv in range(2):
                            nc.tensor.matmul(
                                kp[:, mv * 512:(mv + 1) * 512],
                                lhsT=(wk_sb[:, kc, mc * 128:(mc + 1) * 128]),
                                rhs=(xs[img][:, kc, mv * 512:(mv + 1) * 512]),
                                start=(kc == 0), stop=(kc == NCH - 1))
                    # plain copy: DVE, keeping ACT free for the first exps
                    nc.vector.tensor_copy(k_sb[img][:, mc, :], kp)

            def emit_qkv_v(img, pcs):
                vt4 = vt_sb[img].rearrange("p a (h d) -> p a h d", d=128)
                for pc in pcs:
                    vp = vtps.tile([128, N], F32, tag="vp", bufs=2)
                    for kc in range(NCH):
                        nc.tensor.matmul(
                            vp[:, 0:NH * DH],
                            lhsT=(xs[img][:, kc, pc * 128:(pc + 1) * 128]),
                            rhs=(wv_sb[:, kc, :]),
                            start=(kc == 0), stop=(kc == NCH - 1))
                    for h in range(NH):
                        dst0 = 0 if h % 2 == 0 else 64
                        nc.vector.tensor_copy(
                            vt4[:, pc, h, dst0:dst0 + DH],
                            vp[:, h * DH:(h + 1) * DH])

            return emit_qkv_q, emit_qkv_k, emit_qkv_v

        def make_head(etps, oaps):
            def emit_head(img, h, tail=False, mid=None):
                hc = h // 2
                oaug = oaps.tile([128, N], F32, tag="oaug", bufs=2,
                                 name=f"oaug{img}{h}")
                for pc in range(NNC):
                    et = etps.tile([128, N], F32, tag="et", bufs=2,
                                   name=f"et{pc}")
                    for mv in range(2):
                        nc.tensor.matmul(
                            et[:, mv * 512:(mv + 1) * 512],
                            lhsT=(k_sb[img][(h % 2) * 64:(h % 2) * 64 + 64,
                                            hc, pc * 128:(pc + 1) * 128]),
                            rhs=(q_sb[img][(h % 2) * 64:(h % 2) * 64 + 64,
                                           hc, mv * 512:(mv + 1) * 512]),
                            start=True, stop=True)
                    p_t = work.tile([128, N], BF16, name="p_t", tag="p_t",
                                    bufs=6)
                    nc.scalar.activation(p_t, et,
                                         mybir.ActivationFunctionType.Exp)
                    for mv in range(2):
                        nc.tensor.matmul(
                            oaug[:, mv * 512:(mv + 1) * 512],
                            lhsT=(vt_sb[img][:, pc, h * 128:(h + 1) * 128]),
                            rhs=(p_t[:, mv * 512:(mv + 1) * 512]),
                            start=(pc == 0), stop=(pc == NNC - 1))
                den_chain(img, h, oaug, tail=tail, mid=mid)

            def den_chain(img, h, oaug, tail=False, mid=None):
                hc, ho = h // 2, (h % 2) * 64
                denp = 64 if h % 2 == 0 else 0
                # softmax denominator: row `denp` of oaug.  Reciprocal of
                # the single row first (saves a copy on the critical DVE
                # chain), broadcast across partitions on GPSIMD, then scale
                # the head's 64 rows.  For the chunk-final heads the chain
                # is split into halves so the BN1 stats (and the AllReduce
                # behind them) start sooner, with the residual add moved to
                # GPSIMD so the DVE chain stays short.
                halves = ((0, 512), (512, 1024)) if tail else ((0, 1024),)
                dsb = work.tile([128, N], F32, name="dsb", tag="dsb", bufs=2)
                dbc = work.tile([128, N], F32, name="dbc", tag="dbc", bufs=2)
                rbc = work.tile([128, N], F32, name="rbc", tag="rbc", bufs=2)
                dsb0 = None
                for lo, hi in halves:
                    if denp == 0:
                        # odd heads: denominator row already at partition 0,
                        # so reciprocal the single row first (the custom DVE
                        # op reads absolute partition 0) and broadcast the
                        # result -- one DVE op shorter on the critical chain
                        nc.vector.reciprocal_approx_fast(
                            out=dsb[0:1, lo:hi], in_=oaug[0:1, lo:hi])
                        nc.gpsimd.partition_broadcast(rbc[:, lo:hi],
                                                      dsb[0:1, lo:hi])
                        if mid is not None and lo > 0:
                            # inject independent work (BN1 chunk-0 finish)
                            # into the DVE slot that otherwise idles while
                            # GPSIMD broadcasts the second half
                            mid()
                            mid = None
                    else:
                        # even heads: both the custom reciprocal and
                        # partition_broadcast read ABSOLUTE partition 0 on
                        # HW -- copy the row out of PSUM, bounce it down
                        # with a small SBUF DMA, broadcast, then reciprocal
                        nc.vector.tensor_copy(dsb[denp:denp + 1, lo:hi],
                                              oaug[denp:denp + 1, lo:hi])
                        if dsb0 is None:
                            dsb0 = work.tile([128, N], F32, name="dsb0",
                                             tag="dsb0", bufs=2)
                        nc.gpsimd.dma_start(out=dsb0[0:1, lo:hi],
                                            in_=dsb[denp:denp + 1, lo:hi])
                        nc.gpsimd.partition_broadcast(dbc[:, lo:hi],
                                                      dsb0[0:1, lo:hi])
                        nc.vector.reciprocal_approx_fast(out=rbc[:, lo:hi],
                                                         in_=dbc[:, lo:hi])
                    nc.vector.tensor_mul(o_sb[img][ho:ho + 64, hc, lo:hi],
                                         oaug[ho:ho + 64, lo:hi],
                                         rbc[ho:ho + 64, lo:hi])
                    if h % 2 == 1 and tail:
                        sg = lo // 512
                        nc.gpsimd.tensor_add(
                            o_sb[img][:, hc, lo:hi],
                            o_sb[img][:, hc, lo:hi], xs[img][:, hc, lo:hi])
                        nc.vector.bn_stats(
                            out=st1[:, hc, img * 2 + sg, :],
                            in_=o_sb[img][:, hc, lo:hi])
                if h % 2 == 1 and not tail:
                    # both heads of chunk hc done -> residual + local stats
                    nc.vector.tensor_add(o_sb[img][:, hc, :],
                                         o_sb[img][:, hc, :],
                                         xs[img][:, hc, :])
                    for sg in range(2):
                        nc.vector.bn_stats(
                            out=st1[:, hc, img * 2 + sg, :],
                            in_=o_sb[img][:, hc, sg * 512:(sg + 1) * 512])

            return emit_head

        with tc.tile_pool(name="qkps", bufs=2, space="PSUM") as qkps, \
             tc.tile_pool(name="vtps", bufs=2, space="PSUM") as vtps:
            eq, ek, ev = make_qkv(qkps, vtps)
            for img in range(BL):
                eq(img)
                ek(img)
                ev(img, range(NNC))

        s1_sb = work.tile([128, NCH], F32, name="s1_sb", tag="bns")
        t1_sb = work.tile([128, NCH], F32, name="t1_sb", tag="bnt")
        with tc.tile_pool(name="etps", bufs=2, space="PSUM") as etps, \
             tc.tile_pool(name="oaps", bufs=2, space="PSUM") as oaps:
            eh = make_head(etps, oaps)
            # phase A: channel chunk 0 (heads 0,1) for both images, then
            # fire its stats AllReduce mid-attention (fully hidden)
            eh(0, 0)
            eh(0, 1)
            eh(1, 0)
            eh(1, 1)
            bn_pack("bn1_0", 0, st1, slot=0)
            bn_trigger("bn1_0")
            # phase B: chunk 1 (heads 2,3).  Chunk 0's finish + apply are
            # emitted only after the last head AND after chunk 1's
            # collective fires: its result has been sitting in DRAM since
            # mid-attention, and emitting the dependent ops earlier risks
            # blocking the ACT/DVE streams mid-attention on a slow run.
            def finish_apply_ch0():
                # chunk-0 scale/shift + BN1 apply.  Both images on ACT
                # (idle after the last exp) so the DVE tail chain is not
                # lengthened; its result has been in DRAM since
                # mid-attention, so no engine blocks here.
                bn_finish("bn1_0", [0], s1_sb, t1_sb)
                for img in range(BL):
                    nc.scalar.activation(
                        mh_sb[img][:, 0, :], o_sb[img][:, 0, :],
                        mybir.ActivationFunctionType.Identity,
                        bias=t1_sb[:, 0:1], scale=s1_sb[:, 0:1])

            eh(0, 2)
            eh(0, 3)
            eh(1, 2)
            eh(1, 3, tail=True, mid=finish_apply_ch0)
            bn_pack("bn1_1", 1, st1, slot=1)
            bn_trigger("bn1_1")

            # =========== FFN W1 (inside the attention pools: fp tiles
            # reuse the et/oaug buffer rotation so the first prestart
            # matmuls start the moment the last PV/exp frees a buffer,
            # instead of waiting for the whole pool to drain) ===========
            ffs = [[work.tile([128, N], F16, name=f"ffs{img}_{fc}",
                              tag=f"ffs{img}_{fc}") for fc in range(NFC)]
                   for img in range(BL)]

            fp_n = [0]

            def fp_alloc():
                fp_n[0] += 1
                if fp_n[0] % 2 == 1:
                    fp = etps.tile([128, N], F32, tag="et", bufs=2,
                                   name=f"fp{fp_n[0]}")
                else:
                    fp = oaps.tile([128, N], F32, tag="oaug", bufs=2,
                                   name=f"fp{fp_n[0]}")
                return fp

            # prestart: chunk-0 contribution for the first EIGHT (img, fc)
            # tiles while chunk 1's collective drains.  PSUM only holds 4,
            # so the first 4 close their accumulation group and spill to
            # f16 SBUF (ACT/DVE are idle during the stall); the next 4 stay
            # resident with their group open for the kc=1 continuation.
            spill = [(0, 0), (1, 0), (0, 1), (1, 1)]
            resid = [(0, 2), (1, 2), (0, 3), (1, 3)]
            sp_sb = {}
            fp_pre = {}
            for i, (img, fc) in enumerate(spill):
                fp = fp_alloc()
                for mv in range(2):
                    nc.tensor.matmul(
                        fp[:, mv * 512:(mv + 1) * 512],
                        lhsT=(w1_sb[:, 0, fc * 128:(fc + 1) * 128]),
                        rhs=(mh_sb[img][:, 0, mv * 512:(mv + 1) * 512]),
                        start=True, stop=True)
                sp = work.tile([128, N], F16, name=f"sp{i}", tag=f"sp{i}")
                if i % 2 == 0:
                    nc.scalar.activation(
                        sp, fp, mybir.ActivationFunctionType.Identity)
                else:
                    nc.vector.tensor_copy(sp, fp)
                sp_sb[(img, fc)] = sp
            for img, fc in resid:
                fp = fp_alloc()
                fp_pre[(img, fc)] = fp
                for mv in range(2):
                    nc.tensor.matmul(
                        fp[:, mv * 512:(mv + 1) * 512],
                        lhsT=(w1_sb[:, 0, fc * 128:(fc + 1) * 128]),
                        rhs=(mh_sb[img][:, 0, mv * 512:(mv + 1) * 512]),
                        start=True, stop=False)
            # chunk 1 scale/shift + apply (waits on its collective)
            bn_finish("bn1_1", [1], s1_sb, t1_sb)
            for img in range(BL):
                if img == 0:
                    nc.scalar.activation(
                        mh_sb[img][:, 1, :], o_sb[img][:, 1, :],
                        mybir.ActivationFunctionType.Identity,
                        bias=t1_sb[:, 1:2], scale=s1_sb[:, 1:2])
                else:
                    nc.vector.tensor_scalar(
                        out=mh_sb[img][:, 1, :], in0=o_sb[img][:, 1, :],
                        scalar1=s1_sb[:, 1:2], scalar2=t1_sb[:, 1:2],
                        op0=mybir.AluOpType.mult, op1=mybir.AluOpType.add)

            def finish_w1(img, fc, fp):
                for mv in range(2):
                    nc.tensor.matmul(
                        fp[:, mv * 512:(mv + 1) * 512],
                        lhsT=(w1_sb[:, 1, fc * 128:(fc + 1) * 128]),
                        rhs=(mh_sb[img][:, 1, mv * 512:(mv + 1) * 512]),
                        start=False, stop=True)
                nc.scalar.activation(
                    ffs[img][fc], fp,
                    mybir.ActivationFunctionType.Prelu,
                    bias=b1_sb[:, fc:fc + 1], alpha=a_slope)

            # residents first (their kc=1 continues the open group and
            # frees the PSUM buffers the spilled tiles' kc=1 will rotate
            # into), then the spilled four (fresh psum + add + prelu)
            for img, fc in resid:
                finish_w1(img, fc, fp_pre[(img, fc)])
            for img, fc in spill:
                fq = fp_alloc()
                for mv in range(2):
                    nc.tensor.matmul(
                        fq[:, mv * 512:(mv + 1) * 512],
                        lhsT=(w1_sb[:, 1, fc * 128:(fc + 1) * 128]),
                        rhs=(mh_sb[img][:, 1, mv * 512:(mv + 1) * 512]),
                        start=True, stop=True)
                ft = work.tile([128, N], F16, name=f"ft{img}{fc}",
                               tag="ftmp", bufs=2)
                nc.vector.tensor_add(ft, fq, sp_sb[(img, fc)])
                nc.scalar.activation(
                    ffs[img][fc], ft,
                    mybir.ActivationFunctionType.Prelu,
                    bias=b1_sb[:, fc:fc + 1], alpha=a_slope)
            for img in range(BL):
                for fc in range(4, NFC):
                    fp = fp_alloc()
                    for kc in range(NCH):
                        for mv in range(2):
                            nc.tensor.matmul(
                                fp[:, mv * 512:(mv + 1) * 512],
                                lhsT=(w1_sb[:, kc, fc * 128:(fc + 1) * 128]),
                                rhs=(mh_sb[img][:, kc,
                                                mv * 512:(mv + 1) * 512]),
                                start=(kc == 0), stop=(kc == NCH - 1))
                    nc.scalar.activation(
                        ffs[img][fc], fp,
                        mybir.ActivationFunctionType.Prelu,
                        bias=b1_sb[:, fc:fc + 1], alpha=a_slope)

        # W2, mc-major: chunk 0's stats collective fires after half the
        # work and hides under chunk 1's matmuls
        with tc.tile_pool(name="ops2", bufs=2, space="PSUM") as ops2:
            for mc in range(NCH):
                for img in range(BL):
                    outp = ops2.tile([128, N], F32, tag="outp", bufs=2)
                    for fc in range(NFC):
                        for mv in range(2):
                            nc.tensor.matmul(
                                outp[:, mv * 512:(mv + 1) * 512],
                                lhsT=(w2_sb[:, fc, mc * 128:(mc + 1) * 128]),
                                rhs=(ffs[img][fc][:, mv * 512:(mv + 1) * 512]),
                                start=(fc == 0), stop=(fc == NFC - 1))
                    nc.vector.tensor_add(u_sb[img][:, mc, :], outp,
                                         mh_sb[img][:, mc, :])
                    for sg in range(2):
                        nc.vector.bn_stats(
                            out=st2[:, mc, img * 2 + sg, :],
                            in_=u_sb[img][:, mc, sg * 512:(sg + 1) * 512])
                # pack + ship each chunk as soon as its stats exist (chunk
                # 0's pack/DMA hide under chunk 1's matmuls); one trigger
                # (two collectives in the tail would serialize on the CC
                # stream under peer skew -- measured worse)
                bn_pack("bn2", mc, st2, slot=NCH + mc, nch=NCH, cslot=mc,
                        eng=nc.gpsimd)
            bn_trigger("bn2")

        # =========== BN2 + output (f16, upcast host-side) ===========
        s2_sb = work.tile([128, NCH], F32, name="s2_sb", tag="bns2")
        t2_sb = work.tile([128, NCH], F32, name="t2_sb", tag="bnt2")
        bn_finish("bn2", [0, 1], s2_sb, t2_sb)
        # apply: mh_sb is dead -> reuse as f16 staging; img0 on ACT,
        # img1 on DVE, in half-chunks so the output DMAs start early
        for mc in range(NCH):
            for img in range(BL):
                outr = out_d.ap()[img].rearrange("(c p) n -> p c n", p=128)
                for mv in range(2):
                    sl = slice(mv * 512, (mv + 1) * 512)
                    if img == 0:
                        nc.scalar.activation(
                            mh_sb[img][:, mc, sl], u_sb[img][:, mc, sl],
                            mybir.ActivationFunctionType.Identity,
                            bias=t2_sb[:, mc:mc + 1], scale=s2_sb[:, mc:mc + 1])
                        nc.sync.dma_start(out=outr[:, mc, sl],
                                          in_=mh_sb[img][:, mc, sl])
                    else:
                        nc.vector.tensor_scalar(
                            out=mh_sb[img][:, mc, sl], in0=u_sb[img][:, mc, sl],
                            scalar1=s2_sb[:, mc:mc + 1],
                            scalar2=t2_sb[:, mc:mc + 1],
                            op0=mybir.AluOpType.mult, op1=mybir.AluOpType.add)
                        nc.gpsimd.dma_start(out=outr[:, mc, sl],
                                            in_=mh_sb[img][:, mc, sl])


_COMPILED = None


def _get_compiled(a_slope: float):
    global _COMPILED
    if _COMPILED is None or _COMPILED[0] != a_slope:
        _COMPILED = (a_slope, _build(a_slope))
    return _COMPILED[1]


def _prep_inputs(inputs):
    x = np.ascontiguousarray(np.asarray(inputs["x"], dtype=np.float32))
    Wq = np.asarray(inputs["Wq"], dtype=np.float32)
    Wk = np.asarray(inputs["Wk"], dtype=np.float32)
    Wv = np.asarray(inputs["Wv"], dtype=np.float32)
    bq = np.asarray(inputs["bq"], dtype=np.float32)
    W1 = np.asarray(inputs["W1"], dtype=np.float32)
    b1 = np.asarray(inputs["b1"], dtype=np.float32)
    W2 = np.asarray(inputs["W2"], dtype=np.float32)
    gamma = np.asarray(inputs["gamma"], dtype=np.float32)
    beta = np.asarray(inputs["beta"], dtype=np.float32)

    def pack(mat):
        # [K, M] (K = contraction, rows grouped as (chunk, partition)) ->
        # [128, n_chunks * M] partition-major so the DMA is 128 big rows
        K, M = mat.shape
        return np.ascontiguousarray(
            mat.reshape(K // 128, 128, M).transpose(1, 0, 2).reshape(128, -1)
            .astype(np.float16))

    wvT = np.zeros((C, NH * DH), dtype=np.float32)
    for h in range(NH):
        wvT[:, h * DH:(h + 1) * DH] = Wv[h].T
    sm = np.zeros((128, 14), dtype=np.float32)
    sm[:, 0:NCH] = bq.reshape(NCH, 128).T
    sm[:, NCH:NCH + NFC] = b1.reshape(NFC, 128).T
    sm[:, NCH + NFC:NCH + NFC + NCH] = gamma.reshape(NCH, 128).T
    sm[:, NCH + NFC + NCH:] = beta.reshape(NCH, 128).T
    common = {
        "wqT": pack(Wq.reshape(C, C).T),
        "wkT": pack(Wk.reshape(C, C).T),
        "wvT": pack(wvT),
        "w1T": pack(W1.T),
        "w2T": pack(W2.T),
        "sm": sm,
    }
    # x: [B, C, N] -> per-core [BL, 128, NCH*N] partition-major f16
    xp = np.ascontiguousarray(
        x.reshape(B, NCH, 128, N).transpose(0, 2, 1, 3).reshape(B, 128, NCH * N)
        .astype(np.float16))
    in_maps = []
    for c in range(NCORES):
        m = dict(common)
        m["x"] = np.ascontiguousarray(xp[c * BL:(c + 1) * BL])
        in_maps.append(m)
    return in_maps


def kernel_ex(trace=False, **inputs):
    a_slope = float(np.asarray(inputs["a"]))
    nc = _get_compiled(a_slope)
    in_maps = _prep_inputs(inputs)
    res = bass_utils.run_bass_kernel_spmd(
        nc, in_maps, core_ids=list(range(NCORES)), trace=trace)
    out = np.empty((B, C, N), dtype=np.float32)
    for c in range(NCORES):
        out[c * BL:(c + 1) * BL] = res.results[c]["out"].astype(np.float32)
    return out.reshape(B, C, HH, WW), res


def kernel(**inputs):
    out, _ = kernel_ex(False, **inputs)
    return out
